# revision 1
# baseline (speedup 1.0000x reference)
"""Per-batch covariance + triu gather on 8 Trainium2 NeuronCores.

Problem: inputs [64, 4096, 256] f32 -> out [64, 32896] f32 where
out[b] = triu(cov(inputs[b])) in row-major order and
cov = (xc^T @ xc) / N with xc = x - mean(x, axis=0).

Strategy (data-parallel, 8 batches per core), v3:
- The input DRAM tensor is declared float32r (same bits as f32), so raw
  chunks DMA straight into f32r SBUF tiles and feed single-pass-rate PE
  matmuls with NO DVE pre-pass (v1 spent a ~96us DVE pass rescaling and
  retyping every element; that pass was nearly co-critical with the
  ~94us HBM input stream, which is the roofline for this kernel).
- Per 128-row chunk, three matmuls accumulate in PSUM: G0[0:128,0:256],
  G1[128:256,0:256] (the two row-halves of the unnormalized Gram), and
  s[1,0:256] (column sums, lhsT = ones[128,1]).  Unscaled accumulation
  is safe: |G| <= ~4e3, f32 PSUM has plenty of range.
- Epilogue per batch (all on DVE, which cannot issue DMAs and so never
  steals wave-issue bandwidth): srow=s, nsrow=-s/N, one rank-1 matmul
  per half accumulates -s s^T/N into the same PSUM, and the copy-out
  applies the 1/N scale: cov = (G - s s^T/N)/N.
- triu extraction: 256 row-tail DMAs (one per cov row, covering all 8
  batches each).  dma_start costs ~0.6-0.7us of sequencer time on this
  hardware, so the wave is spread over ALL DMA-capable sequencers
  (SP/ACT/gpsimd; DVE and PE are rejected by bass).  "single" mode
  balances the three; "train2" keeps SP free because in back-to-back
  executions any wave issue on SP queues ahead of the next run's input
  stream and stalls it behind the wave's semaphore wait.
"""

import os
import numpy as np

B, N, D = 64, 4096, 256
NCORES = 8
BPC = B // NCORES          # batches per core
TRI = D * (D + 1) // 2     # 32896
CHUNKS = N // 128          # 32
INV_N = 1.0 / N

TRIU_MODE = os.environ.get("COV_TRIU_MODE", "rowdma")  # "rowdma" | "host"
WAVE_ENGINES = os.environ.get("COV_WAVE_ENGINES", "single")

_cache = {}


def _build(triu_mode, reps=1, variant="base", wave_engines=None):
    import concourse.bacc as bacc
    import concourse.mybir as mybir
    from concourse.tile import TileContext

    F32 = mybir.dt.float32
    F32R = mybir.dt.float32r

    BF16 = mybir.dt.bfloat16
    bf16 = variant == "bf16"

    nc = bacc.Bacc("TRN2", target_bir_lowering=False)
    xdt = mybir.dt.float32 if bf16 else F32R
    x = nc.dram_tensor("x", [BPC, N, D], xdt, kind="ExternalInput")
    if triu_mode == "host":
        out = nc.dram_tensor("out", [BPC, D, D], F32, kind="ExternalOutput")
    else:
        out = nc.dram_tensor("out", [BPC, TRI], F32, kind="ExternalOutput")

    # x[b] rows are assigned to (half, partition, chunk) so each
    # partition's 16 rows are contiguous in DRAM. The contraction over
    # rows is order-invariant, so any bijective row assignment is valid
    # as long as lhsT/rhs read the same tile.
    xv = x.rearrange("b (h p c) d -> b h p c d", h=2, p=128)

    with TileContext(nc) as tc:
        with (
            tc.tile_pool(name="cst", bufs=1) as cst,
            tc.tile_pool(name="xin", bufs=5) as xinp,
            tc.tile_pool(name="sb", bufs=2) as sb,
            tc.tile_pool(name="cov", bufs=2) as covp,
            tc.tile_pool(name="ps", bufs=2, space="PSUM") as ps,
        ):
            ones_f = cst.tile([128, 1], F32)
            nc.vector.memset(ones_f, 1.0)
            ones = cst.tile([128, 1], BF16 if bf16 else F32R)
            nc.scalar.copy(ones, ones_f)

            pstate = {}
            covstate = {}
            wave_engines = wave_engines or WAVE_ENGINES
            lanes = {
                "ss": [nc.sync, nc.scalar],
                "s2": [nc.scalar],
                "y2": [nc.sync],
                "v2": [nc.vector],
                "g2": [nc.gpsimd],
                "t2": [nc.tensor],
                "ssv": [nc.sync, nc.scalar, nc.vector],
                "ssvg": [nc.sync, nc.scalar, nc.vector, nc.gpsimd],
                "ssvt": [nc.sync, nc.scalar, nc.vector, nc.tensor],
                "ssvgt": [nc.sync, nc.scalar, nc.vector, nc.gpsimd,
                          nc.tensor],
                # single-shot: all three DMA-capable sequencers drain the
                # tail wave evenly (sync's input issues are long done)
                "single": [nc.scalar, nc.sync, nc.gpsimd],
                # rep-train: sync still owns the next rep's input issues,
                # so it takes a lighter share of the wave
                "train": [nc.scalar, nc.gpsimd, nc.sync, nc.scalar,
                          nc.gpsimd],
                # rep-train, zero sync share: any wave issue on sync delays
                # the next rep's input stream behind the wave's sem wait
                "train2": [nc.scalar, nc.gpsimd],
            }
            rowdma_engines = lanes[wave_engines]

            # bf16 mode shrinks the lower Gram half to its triu-needed
            # 128 columns (bf16 matmuls run 1 cyc/row at any width)
            bw = 128 if bf16 else 256

            def alloc_cov(rep):
                covA = covp.tile([128, BPC * 256], F32, name=f"cA{rep}",
                                 tag="cA")
                covB = covp.tile([128, BPC * bw], F32, name=f"cB{rep}",
                                 tag="cB")
                covstate[rep] = (covA, covB)
                return covA, covB

            def emit_rowdma_wave(rep, b0, b1):
                covA, covB = covstate.pop(rep)
                covA3 = covA.rearrange("p (b e) -> p b e", e=256)
                covB3 = covB.rearrange("p (b e) -> p b e", e=bw)
                nq = len(rowdma_engines)
                step = 2 if variant == "wavehalf" else 1
                for d in range(0, D, step):
                    p = d % 128
                    ln = D - d
                    off = d * D - (d * (d - 1)) // 2
                    if d < 128:
                        s = covA3[p:p + 1, b0:b1, d:D]
                    else:
                        s = covB3[p:p + 1, b0:b1, d - 256 + bw:bw]
                    dst = out[b0:b1, off:off + ln]  # [b1-b0, ln]
                    rowdma_engines[d % nq].dma_start(dst, s)

            def emit_chunks(key, dma_only=False):
                rep, b = key
                ps0 = ps.tile([128, 256], F32, name=f"ps0_{rep}_{b}", tag="ps0")
                ps1 = ps.tile([128, bw], F32, name=f"ps1_{rep}_{b}", tag="ps1")
                psS = ps.tile([1, 256], F32, name=f"psS_{rep}_{b}", tag="psS")
                halves = []
                for h in range(2):
                    xt = xinp.tile([128, 16 * 256], BF16 if bf16 else F32R,
                                   name=f"xt{rep}_{b}_{h}", tag="xt")
                    xt3 = xt.rearrange("p (c d) -> p c d", d=256)
                    # 4-chunk pieces pipeline PE against the DMA stream; the
                    # final piece of the last batch shrinks to 1 chunk so PE
                    # finishes almost with the stream's last byte
                    if b == BPC - 1 and h == 1:
                        bounds = [0, 4, 8, 12, 15, 16]
                    else:
                        bounds = [0, 4, 8, 12, 16]
                    dma_eng = nc.gpsimd if bf16 else nc.sync
                    for g0, g1 in zip(bounds, bounds[1:]):
                        dma_eng.dma_start(xt3[:, g0:g1, :],
                                          xv[b, h, :, g0:g1, :])
                    halves.append(xt)
                if variant == "dmapure" or dma_only:
                    pstate[key] = (ps0, ps1, psS)
                    return
                for c in range(CHUNKS):
                    xt = halves[c // 16]
                    c0 = (c % 16) * 256
                    sl = xt[:, c0:c0 + 256]
                    lo = xt[:, c0 + 256 - bw:c0 + 256]
                    st = (c == 0)
                    nc.tensor.matmul(ps0, xt[:, c0:c0 + 128], sl, start=st,
                                     stop=False, skip_group_check=True)
                    nc.tensor.matmul(ps1, xt[:, c0 + 128:c0 + 256], lo,
                                     start=st, stop=False,
                                     skip_group_check=True)
                    nc.tensor.matmul(psS, ones, sl, start=st,
                                     stop=(c == CHUNKS - 1),
                                     skip_group_check=True)
                pstate[key] = (ps0, ps1, psS)

            def emit_epilogue(key):
                rep, b = key
                ps0, ps1, psS = pstate.pop(key)
                covA, covB = covstate[rep]
                srow = sb.tile([1, 256], F32R, name=f"sr{rep}_{b}", tag="sr")
                nsrow = sb.tile([1, 256], F32R, name=f"nsr{rep}_{b}", tag="nsr")
                # all epilogue compute on DVE: the DMA-capable sequencers
                # (SP/ACT/gpsimd) stay free for wave issue
                nc.vector.tensor_copy(srow, psS[0:1, :])
                nc.vector.tensor_scalar_mul(nsrow, psS[0:1, :], -INV_N)
                nc.tensor.matmul(ps0, nsrow[0:1, 0:128], srow, start=False,
                                 stop=True, skip_group_check=True)
                nc.tensor.matmul(ps1, nsrow[0:1, 128:256],
                                 srow[0:1, 256 - bw:256], start=False,
                                 stop=True, skip_group_check=True)
                nc.vector.tensor_scalar_mul(covA[:, b * 256:(b + 1) * 256],
                                            ps0, INV_N)
                nc.vector.tensor_scalar_mul(covB[:, b * bw:(b + 1) * bw],
                                            ps1, INV_N)
                if triu_mode == "host":
                    nc.sync.dma_start(out[b, 0:128, :],
                                      covA[:, b * 256:(b + 1) * 256])
                    nc.scalar.dma_start(out[b, 128:256, :],
                                        covB[:, b * 256:(b + 1) * 256])

            if variant in ("waveonly", "waveonly1"):
                covA, covB = alloc_cov(0)
                nc.vector.memset(covA, 0.25)
                nc.vector.memset(covB, 0.25)
                b1 = 1 if variant == "waveonly1" else BPC
                for rep in range(reps):
                    covstate[0] = (covA, covB)
                    emit_rowdma_wave(0, 0, b1)
            elif variant == "dmawave":
                covA, covB = alloc_cov(0)
                nc.vector.memset(covA, 0.25)
                nc.vector.memset(covB, 0.25)
                for rep in range(reps):
                    for b in range(BPC):
                        emit_chunks((rep, b), dma_only=True)
                        pstate.pop((rep, b))
                    covstate[0] = (covA, covB)
                    emit_rowdma_wave(0, 0, BPC)
            else:
                for rep in range(reps):
                    if variant != "dmapure":
                        alloc_cov(rep)
                    for b in range(BPC):
                        emit_chunks((rep, b))
                        if variant == "dmapure":
                            pstate.pop((rep, b))
                            continue
                        if b >= 1:
                            emit_epilogue((rep, b - 1))
                    if variant != "dmapure":
                        emit_epilogue((rep, BPC - 1))
                        if triu_mode == "rowdma" and variant != "nowave":
                            emit_rowdma_wave(rep, 0, BPC)
                        else:
                            covstate.pop(rep)

    nc.finalize()
    return nc


def _get_nc(triu_mode, reps=1, variant="base", wave_engines=None):
    key = (triu_mode, reps, variant, wave_engines or WAVE_ENGINES)
    if key not in _cache:
        _cache[key] = _build(triu_mode, reps, variant, wave_engines)
    return _cache[key]


_TRIU_ROWS = None


def _host_gather(cov_full):
    # cov_full: [B, D, D] -> [B, TRI] row-major upper triangle
    global _TRIU_ROWS
    if _TRIU_ROWS is None:
        _TRIU_ROWS = np.triu_indices(D)
    iu, ju = _TRIU_ROWS
    return cov_full[:, iu, ju]


def kernel(**inputs):
    from concourse.bass_utils import run_bass_kernel_spmd

    x = np.asarray(inputs["inputs"], dtype=np.float32)
    assert x.shape == (B, N, D), x.shape
    nc = _get_nc(TRIU_MODE)
    in_maps = [
        {"x": np.ascontiguousarray(x[c * BPC:(c + 1) * BPC])}
        for c in range(NCORES)
    ]
    res = run_bass_kernel_spmd(nc, in_maps, core_ids=list(range(NCORES)))
    outs = [res.results[c]["out"] for c in range(NCORES)]
    full = np.concatenate(outs, axis=0)
    if TRIU_MODE == "host":
        return _host_gather(full)
    return full.reshape(B, TRI)

